# revision 15
# baseline (speedup 1.0000x reference)
"""Trainium2 Bass kernel for CannyExtractor (NMS-suppressed canny magnitude).

Contract: kernel(x) takes the FULL input x [16,3,512,512] f32 and returns the
FULL output [16,3,512,512] f32. Internally shards the batch over 8 NeuronCores
(2 images per core), runs one SPMD Bass program, and reassembles (channel
replication and fp16 -> fp32 upcast happen host-side).

Final design (measured-cost driven; 207us baseline -> 154us):
  - fp32 head: gray (STT), vertical composite convs on PE (banded matmuls +
    one corner matmul whose rows are permuted so each plane needs only 2
    patch DMAs), horizontal 5/3-tap chains + gradients on DVE full-tile ops.
  - masks (direction bins): fp32 STT diffs + tensor_scalar -> u8.
  - fp16 tail: s is downcast once (ACT); row shifts via fp16 permutation
    matmuls (exact); neighbor maxes / blend / compare / suppression all in
    fp16 (DVE 2x mode). Decision flips vs fp32 cost ~1.5e-2 rel L2 of the
    2e-2 budget (measured offline on the real input distribution).
  - magnitude sqrt reads the fp32 s (value error is only the final fp16
    rounding, ~2e-4 rel).
  - single fp16 output channel; host replicates + upcasts.
  - all input DMAs hoisted ahead of compute; per-half gray so DVE starts
    early; both images' stages emitted breadth-first (engine queues are
    FIFO - a stalled op blocks ready ops queued behind it).
"""
import sys
import os
import numpy as np

sys.path.insert(0, "/opt/trn_rl_repo")

H = W = 512
NT = 4            # 128-row blocks per image
P = 128
WT = 520          # tile width: data cols 4..515, guards either side
OF = 4            # data column offset inside tiles
NI = 2            # images per core
NCORES = 8

GRAY = np.array([0.299, 0.587, 0.114], np.float32)
T2 = np.float32((np.sqrt(2.0) - 1.0) ** 2)   # tan^2(22.5 deg)
EPS = np.float32(1e-6)


def _gauss5():
    ax = np.arange(5, dtype=np.float32) - 2.0
    g1 = np.exp(-0.5 * ax * ax).astype(np.float32)
    return (g1 / g1.sum()).astype(np.float32)


def _vert_matrix(kind):
    """512x512 M[o,i]: vertical composite (3-tap sobel part o replicate-pad o
    gaussian o reflect-pad), float64."""
    g1 = _gauss5()
    I = np.eye(H, dtype=np.float64)
    X = np.pad(I, ((2, 2), (0, 0)), mode="reflect")
    B = np.zeros((H, H))
    for k in range(5):
        B += g1[k] * X[k:k + H]
    Y = np.pad(B, ((1, 1), (0, 0)), mode="edge")
    taps = [1.0, 2.0, 1.0] if kind == "smooth" else [-1.0, 0.0, 1.0]
    M = np.zeros((H, H))
    for k in range(3):
        if taps[k] != 0.0:
            M += taps[k] * Y[k:k + H]
    return M


def _build_consts():
    Ms = (_vert_matrix("smooth") * float(GRAY[2])).astype(np.float32)
    Md = (_vert_matrix("diff") * float(GRAY[2])).astype(np.float32)
    # main block-diagonal bands: vs[k, t, m] = M[128t+m, 128t+k]
    vs = np.zeros((P, NT, P), np.float32)
    vd = np.zeros((P, NT, P), np.float32)
    for t in range(NT):
        vs[:, t, :] = Ms[128 * t:128 * (t + 1), 128 * t:128 * (t + 1)].T
        vd[:, t, :] = Md[128 * t:128 * (t + 1), 128 * t:128 * (t + 1)].T
    # corner matmul, layouts permuted for single-DMA stripping:
    #   cs rows (inputs):  row 3k+b      = g row 122+k of block b   (k<6,b<3)
    #                      row 18+3k+b   = g row k     of block b+1
    #   vcor cols (outputs): col 3i+b    = u row 125+i of block b   (i<3)
    #                        col 9+3i+b  = u row i     of block b+1
    vcor = np.zeros((36, 2, 18), np.float32)
    for b in range(3):
        for k in range(6):
            in_row_lo = 128 * b + 122 + k          # cs row 3k+b
            in_row_hi = 128 * (b + 1) + k          # cs row 18+3k+b
            for i in range(3):
                out_lo = 128 * b + 125 + i         # vcor col 3i+b
                out_hi = 128 * (b + 1) + i         # vcor col 9+3i+b
                vcor[3 * k + b, 0, 3 * i + b] = Ms[out_lo, in_row_lo]
                vcor[3 * k + b, 1, 3 * i + b] = Md[out_lo, in_row_lo]
                vcor[3 * k + b, 0, 9 + 3 * i + b] = Ms[out_hi, in_row_lo]
                vcor[3 * k + b, 1, 9 + 3 * i + b] = Md[out_hi, in_row_lo]
                vcor[18 + 3 * k + b, 0, 3 * i + b] = Ms[out_lo, in_row_hi]
                vcor[18 + 3 * k + b, 1, 3 * i + b] = Md[out_lo, in_row_hi]
                vcor[18 + 3 * k + b, 0, 9 + 3 * i + b] = Ms[out_hi, in_row_hi]
                vcor[18 + 3 * k + b, 1, 9 + 3 * i + b] = Md[out_hi, in_row_hi]
    # shift matrices (fp16-exact 0/1): sup[k,m]=1 iff k=m+1; sdn[k,m]=1 iff k=m-1
    shm = np.zeros((P, 2, P), np.float16)
    for m in range(P - 1):
        shm[m + 1, 0, m] = 1.0
    for m in range(1, P):
        shm[m - 1, 1, m] = 1.0
    return {"vs": vs, "vd": vd, "vcor": vcor, "shm": shm}


_CACHE = {}

# data column ranges inside [P, NT, WT] tiles (data at OF..OF+511)
D0, D1 = OF, OF + W            # 4..516


def _emit_image(nc, tc, pools, tens, img):
    """Generator: yields between pipeline stages so the caller can interleave
    the two images' stages for cross-image engine overlap."""
    import concourse.mybir as mybir
    AL = mybir.AluOpType
    AF = mybir.ActivationFunctionType
    F32 = mybir.dt.float32
    F16 = mybir.dt.float16
    U8 = mybir.dt.uint8

    pwork, ph16, pmask, psmall, ppsum, pcps = pools
    xdram, ydram, c_vs, c_vd, c_vcor, c_shm, zeros16, epsb = tens[:8]

    g1 = _gauss5()
    C0, C1, C2 = float(g1[2]), float(g1[1]), float(g1[0])
    R01 = float(np.float32(GRAY[0] / GRAY[1]))
    R12 = float(np.float32(GRAY[1] / GRAY[2]))

    def wt(name):
        return pwork.tile([P, NT, WT], F32, tag="w", name=name)

    def ht(name):
        return ph16.tile([P, NT, WT], F16, tag="h", name=name)

    def mt(name):
        return pmask.tile([P, NT, WT], U8, tag="m", name=name)

    xc = tens[-1][img]

    # ---- grayscale, per half so it starts when half the DMAs land ----
    gtmp = wt("gtmp")
    g = wt("g")
    for hs in (slice(0, 2), slice(2, 4)):
        nc.vector.affine_then_add(gtmp[:, hs, D0:D1], xc[0][:, hs, D0:D1],
                                  xc[1][:, hs, D0:D1], R01, 0.0)
        nc.vector.affine_then_add(g[:, hs, D0:D1], gtmp[:, hs, D0:D1],
                                  xc[2][:, hs, D0:D1], R12, 0.0)
    yield

    # ---- vertical composite convs on PE; PSUM -> SBUF via ACT copies ----
    u1 = wt("u1")
    u2 = wt("u2")
    for t in range(NT):
        for (cm, u) in ((c_vs, u1), (c_vd, u2)):
            psb = ppsum.tile([P, W], F32, tag="ps", name="vps")
            nc.tensor.matmul(psb[:], cm[:, t, :], g[:, t, D0:D1],
                             start=True, stop=True)
            nc.scalar.activation(u[:, t, D0:D1], psb[:], AF.Copy, 0.0, 1.0)
    # corner strips: 2 DMAs into the permuted [36, W] stack
    cs = psmall.tile([36, W], F32, tag="cs", name="cs")
    nc.sync.dma_start(cs[0:18, :], g[122:128, 0:3, D0:D1])
    nc.sync.dma_start(cs[18:36, :], g[0:6, 1:4, D0:D1])
    for ci, u in ((0, u1), (1, u2)):
        cps = pcps.tile([18, W], F32, tag="cps", name="cps")
        nc.tensor.matmul(cps[:], c_vcor[:, ci, :], cs[:], start=True, stop=True)
        co = psmall.tile([18, W], F32, tag="co", name="co")
        nc.scalar.copy(co[:], cps[:])
        nc.sync.dma_start(u[125:128, 0:3, D0:D1], co[0:9, :])
        nc.sync.dma_start(u[0:3, 1:4, D0:D1], co[9:18, :])
    # reflect guard columns (3 each side) for the horizontal 5-tap window
    for u in (u1, u2):
        nc.scalar.copy(u[:, :, OF - 1:OF], u[:, :, OF + 1:OF + 2])
        nc.scalar.copy(u[:, :, OF - 2:OF - 1], u[:, :, OF + 2:OF + 3])
        nc.scalar.copy(u[:, :, OF - 3:OF - 2], u[:, :, OF + 3:OF + 4])
        nc.scalar.copy(u[:, :, D1:D1 + 1], u[:, :, D1 - 2:D1 - 1])
        nc.scalar.copy(u[:, :, D1 + 1:D1 + 2], u[:, :, D1 - 3:D1 - 2])
        nc.scalar.copy(u[:, :, D1 + 2:D1 + 3], u[:, :, D1 - 4:D1 - 3])
    yield

    # ---- horizontal gaussian (5-tap) at width 514 (cols -1..512) ----
    A0, A1 = OF - 1, OF + 513      # 3..517
    pool_plane = int(os.environ.get("KPOOL", "0"))
    us = (u1, u2)
    a1l = [wt("a1_0"), wt("a1_1")]
    a2l = [wt("a2_0"), wt("a2_1")]
    q1l = [wt("q1_0"), wt("q1_1")]
    bl = [wt("b_0"), wt("b_1")]
    # plane-interleaved emission: DVE works plane 1 while Pool does plane 0
    for pi in (0, 1):
        eng = nc.gpsimd if (pi < pool_plane) else nc.vector
        eng.tensor_tensor(a1l[pi][:, :, A0:A1], us[pi][:, :, A0 - 1:A1 - 1],
                          us[pi][:, :, A0 + 1:A1 + 1], AL.add)
        eng.tensor_tensor(a2l[pi][:, :, A0:A1], us[pi][:, :, A0 - 2:A1 - 2],
                          us[pi][:, :, A0 + 2:A1 + 2], AL.add)
    yield
    for pi in (1, 0):
        nc.vector.affine_then_add(q1l[pi][:, :, A0:A1], a2l[pi][:, :, A0:A1],
                                  a1l[pi][:, :, A0:A1], C2 / C1, 0.0)
    for pi in (1, 0):
        nc.vector.affine_then_add(bl[pi][:, :, A0:A1], q1l[pi][:, :, A0:A1],
                                  us[pi][:, :, A0:A1], C1 / C0, 0.0)
    for pi in (0, 1):
        nc.scalar.copy(bl[pi][:, :, A0:A0 + 1], bl[pi][:, :, A0 + 1:A0 + 2])
        nc.scalar.copy(bl[pi][:, :, A1 - 1:A1], bl[pi][:, :, A1 - 2:A1 - 1])
    b1, b2 = bl
    yield

    # ---- gradients (x1/C0, folded into Square scale), squares, s, masks ----
    gx = wt("gx")
    ay = wt("ay")
    gy = wt("gy")
    sqx = wt("sqx")
    sqy = wt("sqy")
    s = wt("s")
    pxy = wt("pxy")
    chd = wt("chd")
    cvd = wt("cvd")
    md1 = mt("md1")
    chm = mt("chm")
    cvm = mt("cvm")
    nc.vector.tensor_tensor(gx[:, :, D0:D1], b1[:, :, D0 + 1:D1 + 1],
                            b1[:, :, D0 - 1:D1 - 1], AL.subtract)
    nc.vector.tensor_tensor(ay[:, :, D0:D1], b2[:, :, D0 - 1:D1 - 1],
                            b2[:, :, D0 + 1:D1 + 1], AL.add)
    nc.vector.affine_then_add(gy[:, :, D0:D1], b2[:, :, D0:D1],
                              ay[:, :, D0:D1], 2.0, 0.0)
    nc.scalar.activation(sqx[:, :, D0:D1], gx[:, :, D0:D1], AF.Square, 0.0, C0)
    nc.scalar.activation(sqy[:, :, D0:D1], gy[:, :, D0:D1], AF.Square, 0.0, C0)
    nc.vector.tensor_tensor(s[:, :, D0:D1], sqx[:, :, D0:D1],
                            sqy[:, :, D0:D1], AL.add)
    nc.vector.tensor_tensor(pxy[:, :, D0:D1], gx[:, :, D0:D1],
                            gy[:, :, D0:D1], AL.mult)
    nc.vector.tensor_scalar(md1[:, :, D0:D1], pxy[:, :, D0:D1], 0.0,
                            None, AL.is_gt)
    nc.vector.scalar_tensor_tensor(chd[:, :, D0:D1], sqx[:, :, D0:D1],
                                   float(T2), sqy[:, :, D0:D1],
                                   AL.mult, AL.subtract)
    nc.vector.scalar_tensor_tensor(cvd[:, :, D0:D1], sqy[:, :, D0:D1],
                                   float(T2), sqx[:, :, D0:D1],
                                   AL.mult, AL.subtract)
    nc.vector.tensor_scalar(chm[:, :, D0:D1], chd[:, :, D0:D1], 0.0,
                            None, AL.is_ge)
    nc.vector.tensor_scalar(cvm[:, :, D0:D1], cvd[:, :, D0:D1], 0.0,
                            None, AL.is_gt)
    # fp16 compare plane (zero col guards for the +/-1 windows)
    s16 = ht("s16")
    nc.scalar.activation(s16[:, :, D0:D1], s[:, :, D0:D1], AF.Copy, 0.0, 1.0)
    nc.gpsimd.memset(s16[:, :, D0 - 2:D0], 0.0)
    nc.gpsimd.memset(s16[:, :, D1:D1 + 2], 0.0)
    yield

    # ---- fp16 row-shifted planes U[r]=s[r+1], D[r]=s[r-1] on PE ----
    Upl = ht("U")
    Dpl = ht("D")
    for pl in (Upl, Dpl):
        nc.gpsimd.memset(pl[:, :, D0 - 2:D0], 0.0)
        nc.gpsimd.memset(pl[:, :, D1:D1 + 2], 0.0)
    for t in range(NT):
        for (ci, pl) in ((0, Upl), (1, Dpl)):
            psb = ppsum.tile([P, W], F32, tag="ps", name="sps")
            nc.tensor.matmul(psb[:], c_shm[:, ci, :], s16[:, t, D0:D1],
                             start=True, stop=True)
            nc.scalar.activation(pl[:, t, D0:D1], psb[:], AF.Copy, 0.0, 1.0)
    # cross-block rows in 2 DMAs each + zero rows
    nc.sync.dma_start(Upl[127:128, 0:3, D0:D1], s16[0:1, 1:4, D0:D1])
    nc.sync.dma_start(Upl[127:128, 3, D0:D1], zeros16[0:1, 0, :])
    nc.sync.dma_start(Dpl[0:1, 1:4, D0:D1], s16[127:128, 0:3, D0:D1])
    nc.sync.dma_start(Dpl[0:1, 0, D0:D1], zeros16[0:1, 0, :])
    yield

    # ---- fp16 neighbor maxes, axis selection, suppression, output ----
    mh = ht("mh")
    mv = ht("mv")
    dmx = ht("dmx")
    sel = ht("sel")
    mag = ht("mag")
    magc = ht("magc")
    keep = ht("keep")
    out_ = ht("out")
    nc.vector.tensor_tensor(mh[:, :, D0:D1], s16[:, :, D0 - 1:D1 - 1],
                            s16[:, :, D0 + 1:D1 + 1], AL.max)
    nc.vector.tensor_tensor(mv[:, :, D0:D1], Upl[:, :, D0:D1],
                            Dpl[:, :, D0:D1], AL.max)
    nc.vector.tensor_tensor(dmx[:, :, D0:D1], Upl[:, :, D0 + 1:D1 + 1],
                            Dpl[:, :, D0 - 1:D1 - 1], AL.max)
    nc.vector.tensor_tensor(sel[:, :, D0:D1], Upl[:, :, D0 - 1:D1 - 1],
                            Dpl[:, :, D0 + 1:D1 + 1], AL.max)
    nc.vector.copy_predicated(sel[:, :, D0:D1], md1[:, :, D0:D1],
                              dmx[:, :, D0:D1])
    nc.vector.copy_predicated(sel[:, :, D0:D1], cvm[:, :, D0:D1],
                              mv[:, :, D0:D1])
    nc.vector.copy_predicated(sel[:, :, D0:D1], chm[:, :, D0:D1],
                              mh[:, :, D0:D1])
    nc.scalar.activation(mag[:, :, D0:D1], s[:, :, D0:D1], AF.Sqrt,
                         epsb[:], 1.0)
    nc.vector.tensor_scalar(magc[:, :, D0:D1], mag[:, :, D0:D1], 1.0,
                            None, AL.min)
    nc.vector.tensor_tensor(keep[:, :, D0:D1], s16[:, :, D0:D1],
                            sel[:, :, D0:D1], AL.is_gt)
    nc.vector.tensor_tensor(out_[:, :, D0:D1], magc[:, :, D0:D1],
                            keep[:, :, D0:D1], AL.mult)
    nc.sync.dma_start(
        ydram[img, 0].rearrange("(t p) w -> p t w", p=P),
        out_[:, :, D0:D1])
    yield


def _build():
    import concourse.bacc as bacc
    import concourse.mybir as mybir
    from concourse import tile
    F32 = mybir.dt.float32
    F16 = mybir.dt.float16

    nc = bacc.Bacc("TRN2", target_bir_lowering=False, debug=False,
                   num_devices=NCORES)
    xdram = nc.declare_dram_parameter("xc", [NI, 3, H, W], F32, isOutput=False)
    c_vs_d = nc.declare_dram_parameter("vs", [P, NT, P], F32, isOutput=False)
    c_vd_d = nc.declare_dram_parameter("vd", [P, NT, P], F32, isOutput=False)
    c_vcor_d = nc.declare_dram_parameter("vcor", [36, 2, 18], F32, isOutput=False)
    c_shm_d = nc.declare_dram_parameter("shm", [P, 2, P], F16, isOutput=False)
    ydram = nc.declare_dram_parameter("y", [NI, 1, H, W], F16, isOutput=True)

    with tile.TileContext(nc) as tc:
        with tc.tile_pool(name="pconst", bufs=1) as pconst, \
             tc.tile_pool(name="pwork", bufs=16) as pwork, \
             tc.tile_pool(name="ph16", bufs=10) as ph16, \
             tc.tile_pool(name="pmask", bufs=6) as pmask, \
             tc.tile_pool(name="psmall", bufs=2) as psmall, \
             tc.tile_pool(name="ppsum", bufs=6, space="PSUM") as ppsum, \
             tc.tile_pool(name="pcps", bufs=2, space="PSUM") as pcps:
            c_vs = pconst.tile([P, NT, P], F32, tag="cvs")
            nc.sync.dma_start(c_vs[:], c_vs_d[:])
            c_vd = pconst.tile([P, NT, P], F32, tag="cvd")
            nc.sync.dma_start(c_vd[:], c_vd_d[:])
            c_vcor = pconst.tile([36, 2, 18], F32, tag="cvcor")
            nc.sync.dma_start(c_vcor[:], c_vcor_d[:])
            c_shm = pconst.tile([P, 2, P], F16, tag="cshm")
            nc.sync.dma_start(c_shm[:], c_shm_d[:])
            zeros16 = pconst.tile([P, 1, W], F16, tag="zeros16")
            nc.gpsimd.memset(zeros16[:], 0.0)
            epsb = pconst.tile([P, 1], F32, tag="epsb")
            nc.gpsimd.memset(epsb[:], float(EPS))

            pools = (pwork, ph16, pmask, psmall, ppsum, pcps)
            F32_ = mybir.dt.float32
            xc_all = []
            for img in range(NI):
                xci = []
                for c in range(3):
                    t = pwork.tile([P, NT, WT], F32_, tag="w",
                                   name="x%d_%d" % (img, c))
                    for hs in (slice(0, 2), slice(2, 4)):
                        nc.sync.dma_start(
                            t[:, hs, D0:D1],
                            xdram[img, c].rearrange(
                                "(t p) w -> p t w", p=P)[:, hs, :])
                    xci.append(t)
                xc_all.append(xci)
            tens = (xdram, ydram, c_vs, c_vd, c_vcor, c_shm, zeros16, epsb,
                    xc_all)
            gens = [_emit_image(nc, tc, pools, tens, img) for img in range(NI)]
            done = [False] * NI
            # skew image 0 ahead so image 1's DVE stages fill image 0's
            # PE/ACT/DMA stage holes (and vice versa)
            skew = int(os.environ.get("KSKEW", "1"))
            for _ in range(skew):
                try:
                    next(gens[0])
                except StopIteration:
                    done[0] = True
            while not all(done):
                for i, gi in enumerate(gens):
                    if not done[i]:
                        try:
                            next(gi)
                        except StopIteration:
                            done[i] = True

    nc.compile()
    return nc


def _get_nc():
    if "nc" not in _CACHE:
        _CACHE["nc"] = _build()
        _CACHE["consts"] = _build_consts()
    return _CACHE["nc"], _CACHE["consts"]


def kernel(x):
    from concourse.bass_utils import run_bass_kernel_spmd
    x = np.ascontiguousarray(np.asarray(x), dtype=np.float32)
    assert x.shape == (16, 3, H, W), x.shape
    nc, consts = _get_nc()
    in_maps = []
    for c in range(NCORES):
        m = {"xc": x[NI * c:NI * (c + 1)]}
        m.update(consts)
        in_maps.append(m)
    res = run_bass_kernel_spmd(nc, in_maps, list(range(NCORES)))
    y1 = np.concatenate([res.results[c]["y"] for c in range(NCORES)], axis=0)
    return np.repeat(y1.astype(np.float32), 3, axis=1)
